# revision 35
# baseline (speedup 1.0000x reference)
"""Trainium2 Bass kernel for causal cosine-sim multi-head attention.

Reference computation (per batch b of 4, 2048 tokens, dim 1024):
  q,k,v = x @ Wq, x @ Wk, x @ Wv          (inner = 8 heads x 64)
  q,k l2-normalized per head, scale 8.0, causal softmax, out = attn-out @ Wo

Sharding: 8 cores = 4 batches x 2 head-groups (4 heads each).
Core c handles batch c//2, heads [4*(c%2), 4*(c%2)+4).  Each core computes a
partial output (2048, 1024) = o_g @ Wo_g; host sums the two head-group
partials per batch.  No on-chip collectives; the 8 cores run SPMD.

Per-core data flow (matmuls bf16 with fp32 PSUM accumulation):
  x (f32) --cast-DMA--> bf16 --xbar-transpose--> xT [k, tok]
  Qt,Kt = W-tiles.T @ xT          [d, tok] layout (d on partitions)
  norms^2 via self-mul + H-matmul; scales 8/||q||, 1/||k|| via ACT
  abs_reciprocal_sqrt; partition-broadcast through K=1 ones-matmuls;
  Qt *= scale_q, Kt *= scale_k (folds the 8.0 and both l2 norms into S)
  V = xT-tiles.T @ Wv             [tok, d] natural layout, +ones column
  S^T[j,i] = Kt.T @ Qt            per (i-tile 256, j-chunk 128), 4 heads/tile
  A = exp(S^T)  (S bounded in [-8,8]; max-subtraction unnecessary),
  causal mask on diagonal blocks via gpsimd affine_select
  [O^T; sums] = [V|1].T @ A       accumulated over j chunks in PSUM
  normalize by 1/sums = (abs_reciprocal_sqrt(sums))^2, ones-matmul broadcast
  partial = oT-tiles.T @ Wo
"""

import numpy as np

import concourse.bass as bass
import concourse.bacc as bacc
import concourse.mybir as mybir
import concourse.tile as tile
from concourse.bass_utils import run_bass_kernel_spmd

DT = mybir.dt
F32 = DT.float32
BF16 = DT.bfloat16

N_TOK = 2048
DIM = 1024
DG = 256          # inner dims per core (4 heads x 64)
NH = 4            # heads per core
DH = 64
MOUT = 1024


def build_nc(N=N_TOK, phase=4):
    NKC = DIM // 128          # 8 contraction chunks
    NTC = N // 128            # token chunks
    QT = 512                  # qkv-projection token tile
    NQT = N // QT
    IT = 256                  # attention i-tile
    NIT = N // IT
    AF = mybir.ActivationFunctionType

    nc = bacc.Bacc("TRN2", target_bir_lowering=False, debug=False, num_devices=8)
    xt_ext = nc.dram_tensor("xt", [DIM, N], BF16, kind="ExternalInput")
    wq_ext = nc.dram_tensor("wq", [128, DIM // 128, DG], BF16,
                            kind="ExternalInput")
    wk_ext = nc.dram_tensor("wk", [128, DIM // 128, DG], BF16,
                            kind="ExternalInput")
    wv_ext = nc.dram_tensor("wv", [128, DIM // 128, DG], BF16,
                            kind="ExternalInput")
    wo_ext = nc.dram_tensor("wo", [128, DG // 128, MOUT], BF16,
                            kind="ExternalInput")
    out_ext = nc.dram_tensor("out", [MOUT, N], BF16, kind="ExternalOutput")

    with tile.TileContext(nc) as tc:
        with (
            tc.tile_pool(name="persist", bufs=1) as pp,
            tc.tile_pool(name="stage", bufs=3) as st,
            tc.tile_pool(name="attn_sb", bufs=4) as asb,
        ):
            xt = pp.tile([128, NKC, N], BF16, tag="xt")          # x transposed
            wq_sb = pp.tile([128, NKC, DG], BF16, tag="wq")
            wk_sb = pp.tile([128, NKC, DG], BF16, tag="wk")
            wv_sb = pp.tile([128, NKC, DG], BF16, tag="wv")
            wo_sb = pp.tile([128, 2, MOUT], BF16, tag="wo")
            # per-head, base partition 0 (matmul inputs at base>=64 fault on HW)
            qts = pp.tile([64, NH, N], BF16, tag="qts")          # scaled Q^T
            kts = pp.tile([64, NH, N], BF16, tag="kts")          # scaled K^T
            vt = pp.tile([128, NTC, NH, DH + 1], BF16, tag="vt")  # [V | 1]
            ot_raw = pp.tile([128, 2, N], BF16, tag="ot_raw")    # unnormalized O^T
            ot = pp.tile([128, 2, N], BF16, tag="ot")            # normalized O^T
            # scale rows: rows 0..31 (dup) = even head, row 32 = odd head
            rq_dc = [pp.tile([33, N], BF16, tag=f"rq{dc}", name=f"rq{dc}")
                     for dc in range(2)]
            rk_dc = [pp.tile([33, N], BF16, tag=f"rk{dc}", name=f"rk{dc}")
                     for dc in range(2)]
            # softmax denominators / their rsqrt, packed 2 heads per tile
            # (rows 0 and 32 -- matmul rhs base partitions must be 32-aligned)
            sums_p = [pp.tile([33, N], F32, tag=f"sums{p}", name=f"sums{p}")
                      for p in range(2)]
            rsq_p = [pp.tile([33, N], BF16, tag=f"rsq{p}", name=f"rsq{p}")
                     for p in range(2)]
            ones_bf = pp.tile([33, 64], BF16, tag="ones_bf")
            hmat = pp.tile([128, 33], BF16, tag="hmat")  # head-sum matrix

            # causal mask tiles for the 4 diagonal 128-offsets (i-tile 512)
            masks = [pp.tile([128, 512], BF16, tag=f"mask{v}", name=f"mask{v}")
                     for v in range(4)]
            onesb = pp.tile([128, 512], BF16, tag="onesb")
            # constants
            nc.vector.memset(hmat[:, :], 0.0)
            nc.vector.memset(hmat[0:64, 0:32], 1.0)   # cols 0..31: even head
            nc.vector.memset(hmat[64:128, 32:33], 1.0)  # col 32: odd head
            nc.vector.memset(ones_bf[:, :], 1.0)
            nc.vector.memset(onesb[:, :], 1.0)
            for p_ in range(2):
                nc.vector.memset(sums_p[p_][:, :], 1.0)
            for v in range(4):
                # keep where j <= i  <=>  f - p - 128*v >= 0
                nc.gpsimd.affine_select(
                    masks[v][:, :], onesb[:, :], pattern=[[1, 512]],
                    compare_op=mybir.AluOpType.is_ge, fill=0.0,
                    base=-128 * v, channel_multiplier=-1)

            # weights arrive pre-marshalled ([128, c, n] bf16) from the host
            nc.sync.dma_start(wq_sb[:, :, :], wq_ext.ap())
            nc.scalar.dma_start(wk_sb[:, :, :], wk_ext.ap())
            nc.sync.dma_start(wv_sb[:, :, :], wv_ext.ap())
            nc.scalar.dma_start(wo_sb[:, :, :], wo_ext.ap())

            # x arrives pre-transposed and pre-cast to bf16 from the host.
            # Load in column halves, first-half chunks first: tile-0 QKV only
            # needs columns 0..511 of every k-chunk, so it can start early.
            for ch in range(2):
                csl = slice(ch * (N // 2), (ch + 1) * (N // 2))
                for kc in range(NKC):
                    eng = nc.sync if kc % 2 == 0 else nc.scalar
                    eng.dma_start(xt[:, kc, csl],
                                  xt_ext[kc * 128:(kc + 1) * 128, csl])

            # ---- QKV projections + l2-norm scales ----
            with tc.tile_pool(name="psA", bufs=2, space="PSUM") as psA:
                for t in range(NQT if phase >= 2 else 0):
                    tsl = slice(t * QT, (t + 1) * QT)
                    for wsb, dst, rdc, sqscale in (
                        (wq_sb, qts, rq_dc, 1.0 / 64.0),  # arsqrt(nq/64)=8/||q||
                        (wk_sb, kts, rk_dc, 1.0),
                    ):
                        for dc in range(2):
                            pps = psA.tile([128, QT], F32, tag="qk_ps", bufs=4)
                            for kc in range(NKC):
                                nc.tensor.matmul(
                                    pps[:, :],
                                    wsb[:, kc, dc * 128:(dc + 1) * 128],
                                    xt[:, kc, tsl],
                                    start=(kc == 0), stop=(kc == NKC - 1))
                            qsb = st.tile([128, QT], F32, tag="qsb", bufs=4)
                            nc.vector.tensor_copy(qsb[:, :], pps[:, :])
                            sq = st.tile([128, QT], BF16, tag="sq", bufs=3)
                            nc.vector.tensor_mul(sq[:, :], qsb[:, :], qsb[:, :])
                            nps = psA.tile([33, QT], F32, tag="norm_ps", bufs=1)
                            nc.tensor.matmul(nps[:, :], hmat[:, :], sq[:, :],
                                             start=True, stop=True)
                            # rows 0..31 = 1/||even||, row 32 = 1/||odd||
                            nc.scalar.activation(rdc[dc][:, tsl], nps[:, :],
                                                 AF.Abs_reciprocal_sqrt,
                                                 scale=sqscale)
                            bc_ps = psA.tile([128, QT], F32, tag="bc_ps")
                            for half in range(2):
                                nc.tensor.matmul(
                                    bc_ps[64 * half:64 * half + 64, :],
                                    ones_bf[32 * half:32 * half + 1, :],
                                    rdc[dc][32 * half:32 * half + 1, tsl],
                                    start=True, stop=True)
                            for half in range(2):
                                pr = 64 * half
                                nc.vector.tensor_mul(
                                    dst[0:64, 2 * dc + half, tsl],
                                    qsb[pr:pr + 64, :], bc_ps[pr:pr + 64, :])
                    # V for the 4 token-chunks of this tile, with ones column
                    for tcc in range(4 * t, 4 * t + 4):
                        vps = psA.tile([128, DG], F32, tag="v_ps", bufs=1)
                        for kc in range(NKC):
                            nc.tensor.matmul(
                                vps[:, :],
                                xt[:, kc, tcc * 128:(tcc + 1) * 128],
                                wv_sb[:, kc, :],
                                start=(kc == 0), stop=(kc == NKC - 1))
                        nc.vector.tensor_copy(
                            vt[:, tcc, :, 0:64],
                            vps[:, :].rearrange("p (h d) -> p h d", d=64))
                        nc.vector.memset(vt[:, tcc, :, 64:65], 1.0)

            # ---- attention ----
            with (
                tc.tile_pool(name="psS", bufs=2, space="PSUM") as psS,
                tc.tile_pool(name="psO", bufs=1, space="PSUM") as psO,
            ):
                for t in range(NQT if phase >= 3 else 0):
                    isl = slice(t * QT, (t + 1) * QT)
                    o_ps = [psO.tile([65, QT], F32, tag=f"o_ps{h}",
                                     name=f"o_ps{h}_{t}")
                            for h in range(NH)]
                    njc = 4 * (t + 1)
                    for jc in range(njc):
                        diag = jc - 4 * t
                        for pair in range(2):
                            s2 = psS.tile([128, 2 * QT], F32, tag="s2")
                            for hh in range(2):
                                h = 2 * pair + hh
                                nc.tensor.matmul(
                                    s2[:, hh * QT:(hh + 1) * QT],
                                    kts[0:64, h, jc * 128:(jc + 1) * 128],
                                    qts[0:64, h, isl],
                                    start=True, stop=True)
                            a2 = asb.tile([128, 2 * QT], BF16, tag="a2", bufs=6)
                            nc.scalar.activation(a2[:, :], s2[:, :], AF.Exp)
                            if diag >= 0:
                                am = asb.tile([128, 2 * QT], BF16, tag="am", bufs=6)
                                for hh in range(2):
                                    nc.vector.tensor_mul(
                                        am[:, hh * QT:(hh + 1) * QT],
                                        a2[:, hh * QT:(hh + 1) * QT],
                                        masks[diag][:, :])
                                a_use = am
                            else:
                                a_use = a2
                            for hh in range(2):
                                h = 2 * pair + hh
                                nc.tensor.matmul(
                                    o_ps[h][:, :], vt[:, jc, h, :],
                                    a_use[:, hh * QT:(hh + 1) * QT],
                                    start=(jc == 0), stop=(jc == njc - 1))
                    for h in range(NH):
                        dc, half = divmod(h, 2)
                        pr = 64 * half
                        nc.vector.tensor_copy(
                            sums_p[dc][32 * half:32 * half + 1, isl],
                            o_ps[h][64:65, :])
                        nc.vector.tensor_copy(
                            ot_raw[pr:pr + 64, dc, isl],
                            o_ps[h][0:64, :])

            # ---- normalize O and output projection ----
            with tc.tile_pool(name="psC", bufs=2, space="PSUM") as psC:
                # one arsqrt per tile: ACT cost is free-size bound, so
                # [33, N] costs the same as [1, N] (rows 1..31 are dummy 1.0)
                for dc in range(2 if phase >= 4 else 0):
                    nc.scalar.activation(rsq_p[dc][:, :], sums_p[dc][:, :],
                                         AF.Abs_reciprocal_sqrt)
                for t in range(NQT if phase >= 4 else 0):
                    tsl = slice(t * QT, (t + 1) * QT)
                    for dc in range(2):
                        bco_ps = psC.tile([128, QT], F32, tag="bco_ps")
                        for half in range(2):
                            r0 = 32 * half
                            nc.tensor.matmul(
                                bco_ps[64 * half:64 * half + 64, :],
                                ones_bf[r0:r0 + 1, :],
                                rsq_p[dc][r0:r0 + 1, tsl],
                                start=True, stop=True)
                        bco = st.tile([128, QT], F32, tag="bco", bufs=3)
                        # square on copy-out: (1/sqrt(s))^2 = 1/s, broadcast
                        nc.scalar.square(bco[:, :], bco_ps[:, :])
                        nc.vector.tensor_mul(ot[:, dc, tsl],
                                             ot_raw[:, dc, tsl], bco[:, :])
                for mc in range(MOUT // 128):
                    osb = st.tile([128, N], BF16, tag="osb", bufs=2)
                    pps_l = [psC.tile([128, 512], F32, tag="op_ps", bufs=6,
                                      name=f"op_{mc}_{t}")
                             for t in range(NQT)]
                    # dc outer: 4 consecutive matmuls share each wo tile
                    for dc in range(2):
                        for t in range(NQT):
                            nc.tensor.matmul(
                                pps_l[t][:, :],
                                wo_sb[:, dc, mc * 128:(mc + 1) * 128],
                                ot[:, dc, t * QT:(t + 1) * QT],
                                start=(dc == 0), stop=(dc == 1))
                    for t in range(NQT):
                        tsl = slice(t * QT, (t + 1) * QT)
                        if t % 2 == 0:
                            nc.scalar.copy(osb[:, tsl], pps_l[t][:, :])
                        else:
                            nc.vector.tensor_copy(osb[:, tsl], pps_l[t][:, :])
                    nc.sync.dma_start(out_ext[mc * 128:(mc + 1) * 128, :],
                                      osb[:, :])

    nc.compile()
    return nc


_NC_CACHE = {}


def _get_nc(N=N_TOK):
    if N not in _NC_CACHE:
        _NC_CACHE[N] = build_nc(N)
    return _NC_CACHE[N]


def _marshal_w(w):
    """[c*128, n] -> [128, c, n] bf16 contiguous (device SBUF layout)."""
    c = w.shape[0] // 128
    return np.ascontiguousarray(
        w.reshape(c, 128, -1).transpose(1, 0, 2)).astype(mybir.dt.np(BF16))


def make_in_maps(x, Wq, Wk, Wv, Wo):
    in_maps = []
    for c in range(8):
        b, g = divmod(c, 2)
        gsl = slice(g * DG, (g + 1) * DG)
        in_maps.append({
            "xt": np.ascontiguousarray(x[b].T).astype(mybir.dt.np(BF16)),
            "wq": _marshal_w(Wq[:, gsl]),
            "wk": _marshal_w(Wk[:, gsl]),
            "wv": _marshal_w(Wv[:, gsl]),
            "wo": _marshal_w(Wo[gsl, :]),
        })
    return in_maps


def kernel(x, Wq, Wk, Wv, Wo, _trace=False):
    x = np.asarray(x)
    nc = _get_nc(x.shape[1])
    in_maps = make_in_maps(np.asarray(x), np.asarray(Wq), np.asarray(Wk),
                           np.asarray(Wv), np.asarray(Wo))
    res = run_bass_kernel_spmd(nc, in_maps, core_ids=list(range(8)),
                               trace=_trace)
    kernel.last_results = res
    out = np.empty((x.shape[0], x.shape[1], MOUT), dtype=np.float32)
    for b in range(x.shape[0]):
        a = res.results[2 * b]["out"].astype(np.float32)
        c = res.results[2 * b + 1]["out"].astype(np.float32)
        out[b] = (a + c).T
    return out
